# revision 10
# baseline (speedup 1.0000x reference)
"""Trainium2 Bass kernel for nn_MemoryAccessWithUserEmbedding.

Sharding: data-parallel over batch across 8 cores (32 samples/core).
Host prep (inside kernel(), layout/gather only + constant folding of
W_proc into the per-shard Wq tail):

  q[s,o]   = x[b] @ Wq_x[sid] + ue[b] @ (W_proc @ Wq_pe[sid]) + b_proc @ Wq_pe[sid]
           = sum_c xe[b,c].T @ wqe[b,c]          (3 K-chunks of 128)
  scoresT  = memT[sid] chunkwise  @ qT           ([m,(r,s)] layout, m on partitions)
  expT     = exp(scoresT)                        (no max-subtraction: |scores| < ~45)
  readT'   = [mem[sid] | 1] chunkwise @ expT     (ones column gives softmax denom)
  read     = transpose(readT') * recip(denom)    (per (s,r) scale)
  final_state = transpose(expT) * recip(denom)   (last sample of core 7 == batch 255)
"""

import numpy as np

B = 256
S = 128
INPUT_SIZE = 256
PROC_DIM = 128
NUM_SHARDS = 16
MEM_SIZE = 512
WORD_SIZE = 64
NUM_READS = 4
N_CORES = 8
BPC = B // N_CORES  # samples per core
O = NUM_READS * WORD_SIZE  # 256
F32 = None  # set after mybir import

_NC_CACHE = {}
LAST_RESULTS = None


def _build_nc(n_samples=BPC):
    import concourse.bass as bass
    import concourse.tile as tile
    from concourse import mybir
    from concourse.masks import make_identity

    f32 = mybir.dt.float32
    nc = bass.Bass()

    xe = nc.dram_tensor("xe", [n_samples, 3, 128, 128], f32, kind="ExternalInput")
    wqe = nc.dram_tensor("wqe", [n_samples, 3, 128, O], f32, kind="ExternalInput")
    memt1 = nc.dram_tensor("memt1", [n_samples, 64, MEM_SIZE], f32, kind="ExternalInput")
    meme = nc.dram_tensor("meme", [n_samples, 4, 128, 65], f32, kind="ExternalInput")
    out_rw = nc.dram_tensor("out_rw", [n_samples, S, O], f32, kind="ExternalOutput")
    out_fs = nc.dram_tensor("out_fs", [S, NUM_READS * MEM_SIZE], f32, kind="ExternalOutput")

    with tile.TileContext(nc) as tc:
        with (
            tc.tile_pool(name="io", bufs=3) as io,
            tc.tile_pool(name="work", bufs=2) as work,
            tc.tile_pool(name="expp", bufs=2) as expp,
            tc.tile_pool(name="outp", bufs=3) as outp,
            tc.tile_pool(name="const", bufs=1) as const,
            tc.tile_pool(name="fsp", bufs=1) as fsp,
            tc.tile_pool(name="ps_qt", bufs=2, space=bass.MemorySpace.PSUM) as ps_qt,
            tc.tile_pool(name="ps_sc", bufs=2, space=bass.MemorySpace.PSUM) as ps_sc,
            tc.tile_pool(name="ps_rd", bufs=1, space=bass.MemorySpace.PSUM) as ps_rd,
            tc.tile_pool(name="ps_rdt", bufs=1, space=bass.MemorySpace.PSUM) as ps_rdt,
        ):
            id_t = const.tile([128, 128], f32)
            make_identity(nc, id_t)

            # persistent memT slots [128, 2, 512]: [:, 0] = [memT; 0] for even r,
            # [:, 1] = [0; memT] for odd r. Zero halves are memset once; per-sample
            # DMAs only overwrite the memT halves, so zeros persist across reuse.
            # (Row-tiled matmuls at base_partition 64 fail at runtime here, so
            # parity selection is done with zero-padded K=128 stationaries.)
            NMZ = 3
            mz_slots = []
            for k in range(NMZ):
                mz = const.tile([128, 2, MEM_SIZE], f32, tag=f"mz{k}")
                nc.vector.memset(mz[64:128, 0, :], 0.0)
                nc.vector.memset(mz[0:64, 1, :], 0.0)
                mz_slots.append(mz)

            for i in range(n_samples):
                # ---- loads ----
                xe_t = io.tile([128, 3, 128], f32, tag="xe")
                for c in range(3):
                    nc.sync.dma_start(out=xe_t[:, c, :], in_=xe[i, c])
                wqe_t = io.tile([128, 3, O], f32, tag="wqe")
                for c in range(3):
                    nc.sync.dma_start(out=wqe_t[:, c, :], in_=wqe[i, c])
                mz_t = mz_slots[i % NMZ]
                nc.sync.dma_start(out=mz_t[0:64, 0, :], in_=memt1[i])
                nc.sync.dma_start(out=mz_t[64:128, 1, :], in_=memt1[i])
                meme_t = io.tile([128, 4, 65], f32, tag="meme")
                for mc in range(4):
                    nc.sync.dma_start(out=meme_t[:, mc, :], in_=meme[i, mc])

                # ---- qT = WqE.T @ xeT : [o, s] as [128, 2, 128] (o-half on partitions) ----
                qT_ps = ps_qt.tile([128, 2, 128], f32, tag="qT")
                for h in range(2):
                    for c in range(3):
                        nc.tensor.matmul(
                            qT_ps[:, h, :],
                            wqe_t[:, c, h * 128 : (h + 1) * 128],
                            xe_t[:, c, :],
                            start=(c == 0),
                            stop=(c == 2),
                        )
                qT_sb = work.tile([128, 2, 128], f32, tag="qT_sb")
                nc.vector.tensor_copy(qT_sb, qT_ps)

                # ---- scoresT = memT @ q per (mc, r): [m, (r,s)]; r pairs row-tiled ----
                exp_sb = expp.tile([128, 4, 512], f32, tag="exp")
                for h in range(2):
                    sc_ps = ps_sc.tile([128, 2, 512], f32, tag="sc")
                    for j in range(2):
                        mc = 2 * h + j
                        mcs = slice(mc * 128, (mc + 1) * 128)
                        for r in range(4):
                            hr, rp = r // 2, r % 2
                            nc.tensor.matmul(
                                sc_ps[:, j, r * 128 : (r + 1) * 128],
                                mz_t[:, rp, mcs],
                                qT_sb[:, hr, :],
                                start=True,
                                stop=True,
                            )
                    # exp over this half: [128, 1024]
                    nc.scalar.activation(
                        exp_sb[:, 2 * h : 2 * h + 2, :],
                        sc_ps[:, :, :],
                        mybir.ActivationFunctionType.Exp,
                    )

                # ---- readT' = [mem|1].T-chunks @ expT : [65, (r,s)] ----
                rd_ps = ps_rd.tile([65, 512], f32, tag="rd")
                for mc in range(4):
                    nc.tensor.matmul(
                        rd_ps,
                        meme_t[:, mc, :],
                        exp_sb[:, mc, :],
                        start=(mc == 0),
                        stop=(mc == 3),
                    )
                rd_sb = work.tile([65, 512], f32, tag="rd_sb")
                nc.vector.tensor_copy(rd_sb, rd_ps)

                # ---- transpose per r: [65,128] -> [128,65]; then normalize ----
                rdT_ps = ps_rdt.tile([128, 4, 65], f32, tag="rdt")
                for r in range(4):
                    nc.tensor.transpose(
                        rdT_ps[:, r, :],
                        rd_sb[0:65, r * 128 : (r + 1) * 128],
                        id_t[0:65, 0:65],
                    )
                recip_sb = work.tile([128, 4], f32, tag="recip")
                nc.vector.reciprocal(recip_sb, rdT_ps[:, :, 64])
                out_sb = outp.tile([128, 4, 64], f32, tag="out")
                for r in range(4):
                    nc.vector.tensor_scalar_mul(
                        out=out_sb[:, r, :],
                        in0=rdT_ps[:, r, 0:64],
                        scalar1=recip_sb[:, r : r + 1],
                    )
                nc.sync.dma_start(out=out_rw[i], in_=out_sb)

                # ---- final_state for the last sample ----
                if i == n_samples - 1:
                    fs_sb = fsp.tile([128, 4, 4, 128], f32, tag="fs")
                    for r in range(4):
                        for mc in range(4):
                            fsT_ps = ps_qt.tile([128, 128], f32, tag="qT")
                            nc.tensor.transpose(
                                fsT_ps,
                                exp_sb[:, mc, r * 128 : (r + 1) * 128],
                                id_t,
                            )
                            nc.vector.tensor_scalar_mul(
                                out=fs_sb[:, r, mc, :],
                                in0=fsT_ps,
                                scalar1=recip_sb[:, r : r + 1],
                            )
                    nc.sync.dma_start(out=out_fs[:], in_=fs_sb)

    _install_legalize(nc)
    return nc


def _install_legalize(nc):
    """Hoist attached semaphore waits onto standalone EventSemaphore
    instructions: walrus codegen in this toolchain rejects any wait attached
    to a PE Matmult (and >1 wait on most other instructions) with "Too many
    sync wait commands". Raw bass encodes waits standalone; mimic that."""
    import json as _json

    _SYNC_OPS = {
        "EventSemaphore",
        "NoOp",
        "UnconditionalBranch",
        "ConditionalBranch",
        "Call",
        "Return",
        "IncSwdgeSem",
    }
    _KEEP_ONE = {"Drain"}
    counter = [0]

    def hoist(block):
        out = []
        changed = False
        for ins in block.get("instructions", []):
            si = ins.get("sync_info")
            waits = (si or {}).get("on_wait") or []
            op = ins.get("opcode")
            keep = 1 if op in _KEEP_ONE else 0
            if len(waits) > keep and op not in _SYNC_OPS:
                hoisted = waits[: len(waits) - keep]
                kept = waits[len(waits) - keep :]
                for w in hoisted:
                    counter[0] += 1
                    out.append(
                        {
                            "debug": ins.get("debug", 0),
                            "engine": ins["engine"],
                            "ins": [],
                            "outs": [],
                            "name": f"hoistw_{counter[0]}",
                            "opcode": "EventSemaphore",
                            "sync_info": {"on_update": [], "on_wait": [w]},
                        }
                    )
                si["on_wait"] = kept
                changed = True
            out.append(ins)
        if changed:
            block["instructions"] = out
        for sub in block.get("blocks", []):
            hoist(sub)

    orig = nc.to_json_bytes

    def patched(*a, **k):
        bir = _json.loads(orig(*a, **k))
        for fn in bir.get("functions", []):
            for b in fn.get("blocks", []):
                hoist(b)
        return _json.dumps(bir).encode()

    nc.to_json_bytes = patched
    return nc


def _prep_inputs(x, user_id, user_emb_table, W_proc, b_proc, Wq, memory):
    x = np.asarray(x, dtype=np.float32)
    user_id = np.asarray(user_id)
    user_emb_table = np.asarray(user_emb_table, dtype=np.float32)
    W_proc = np.asarray(W_proc, dtype=np.float32)
    b_proc = np.asarray(b_proc, dtype=np.float32)
    Wq = np.asarray(Wq, dtype=np.float32)
    memory = np.asarray(memory, dtype=np.float32)

    sid = (user_id % NUM_SHARDS).astype(np.int64)
    ue = user_emb_table[user_id]  # [B, 64]

    # fold W_proc/b_proc into the per-shard Wq tail
    Wq_x = Wq[:, :INPUT_SIZE, :]  # [16, 256, 256]
    Wq_pe = Wq[:, INPUT_SIZE:, :]  # [16, 128, 256]
    Wq_eff = np.einsum("ep,spo->seo", W_proc, Wq_pe)  # [16, 64, 256]
    c_b = np.einsum("p,spo->so", b_proc, Wq_pe)  # [16, 256]
    WqE = np.zeros((NUM_SHARDS, 3, 128, O), dtype=np.float32)
    WqE[:, 0] = Wq_x[:, 0:128]
    WqE[:, 1] = Wq_x[:, 128:256]
    WqE[:, 2, 0:64] = Wq_eff
    WqE[:, 2, 64] = c_b

    memT = memory.transpose(0, 2, 1)  # [16, 64, 512]
    memE = np.zeros((NUM_SHARDS, 4, 128, 65), dtype=np.float32)
    memE[..., :64] = memory.reshape(NUM_SHARDS, 4, 128, WORD_SIZE)
    memE[..., 64] = 1.0

    xt = x.transpose(0, 2, 1)  # [B, 256, 128]
    xe_all = np.zeros((B, 3, 128, 128), dtype=np.float32)
    xe_all[:, 0] = xt[:, 0:128]
    xe_all[:, 1] = xt[:, 128:256]
    xe_all[:, 2, 0:64] = np.broadcast_to(ue[:, :, None], (B, 64, 128))
    xe_all[:, 2, 64] = 1.0

    in_maps = []
    for c in range(N_CORES):
        sl = slice(c * BPC, (c + 1) * BPC)
        sid_c = sid[sl]
        in_maps.append(
            {
                "xe": np.ascontiguousarray(xe_all[sl]),
                "wqe": np.ascontiguousarray(WqE[sid_c]),
                "memt1": np.ascontiguousarray(memT[sid_c]),
                "meme": np.ascontiguousarray(memE[sid_c]),
            }
        )
    return in_maps


def kernel(x, user_id, user_emb_table, W_proc, b_proc, Wq, memory, _trace=False):
    global LAST_RESULTS
    from concourse.bass_utils import run_bass_kernel_spmd

    if "nc" not in _NC_CACHE:
        _NC_CACHE["nc"] = _build_nc()
    nc = _NC_CACHE["nc"]

    in_maps = _prep_inputs(x, user_id, user_emb_table, W_proc, b_proc, Wq, memory)
    res = run_bass_kernel_spmd(nc, in_maps, core_ids=list(range(N_CORES)), trace=_trace)
    LAST_RESULTS = res

    rw = np.concatenate([r["out_rw"] for r in res.results], axis=0)  # [B, S, 256]
    read_words = rw.reshape(B, S, NUM_READS, WORD_SIZE)
    final_state = res.results[N_CORES - 1]["out_fs"].reshape(S, NUM_READS, MEM_SIZE)
    return read_words, final_state
